# revision 24
# baseline (speedup 1.0000x reference)
"""Multi-head causal attention (B=2, T=2048, C=1024, H=16) on 8 trn2 NeuronCores.

Sharding: 2 heads per core (tensor-parallel over heads), both batch elements
on every core. Per core:
  1. qkv projection for its 2 heads. q^T,k^T run in fp8e4m3 DoubleRow mode
     (x and W_q/W_k supplied fp8 by the host, W pre-scaled by 32 to dodge
     fp8 denormals; the 1/1024 comes out in the exp scale). v runs in fp16.
     q^T,k^T are evicted to fp8 and repacked (SBUF->SBUF DMA) into DoubleRow
     layout [32, 2*T] so S = K^T Q also runs fp8 DoubleRow (2x PE rate).
  2. Flash-style causal attention in the S^T = k q^T layout: exp on ScalarE
     straight out of PSUM, row-sums folded into the P@V matmul via a ones
     column in V, reciprocal multiply on VectorE.
  3. The output exchange is split into 4 AllToAlls (one per 1024 global
     rows), each fired as soon as its two 512-query chunks finish; the
     per-group output projection is interleaved into the remaining
     attention compute. Core c owns rows 1024g+128c..+128 of group g.
Host side shards/transposes/casts inputs and reassembles the output.
"""

import sys

import numpy as np

if "/opt/trn_rl_repo" not in sys.path:
    sys.path.insert(0, "/opt/trn_rl_repo")

B, T, C, H, D = 2, 2048, 1024, 16, 64
NCORES = 8
HPC = H // NCORES          # heads per core = 2
CW = HPC * D               # per-core channel width = 128
KT = C // 128              # k tiles = 8
TT = T // 128              # t tiles = 16
SHARD = (B * T) // NCORES  # output rows per core = 512
WSCALE = 32.0              # host pre-scale on W_q/W_k (fp8 denormal dodge)
SCALE = 1.0 / float(np.sqrt(C))
SCALE_S = SCALE / (WSCALE * WSCALE)

_CACHE = {}
LAST_EXEC_NS = None
_SENTINEL = object()


def _build_nc():
    import concourse.mybir as mybir
    import concourse.tile as tile
    from concourse import bacc
    from concourse.masks import make_identity, make_upper_triangular

    f32 = mybir.dt.float32
    f16 = mybir.dt.float16
    f8 = mybir.dt.float8e4

    nc = bacc.Bacc("TRN2", target_bir_lowering=False, debug=False,
                   num_devices=NCORES)

    xT = nc.dram_tensor("xT", [B, 128, KT * T], f16, kind="ExternalInput")
    x8 = nc.dram_tensor("x8", [B, 128, KT * T], f8, kind="ExternalInput")
    wq = nc.dram_tensor("wq", [128, KT * CW], f8, kind="ExternalInput")
    wk = nc.dram_tensor("wk", [128, KT * CW], f8, kind="ExternalInput")
    wv = nc.dram_tensor("wv", [128, KT * CW], f16, kind="ExternalInput")
    wp = nc.dram_tensor("wp", [128, KT * C], f16, kind="ExternalInput")
    bq = nc.dram_tensor("bq", [CW, 1], f32, kind="ExternalInput")
    bk = nc.dram_tensor("bk", [CW, 1], f32, kind="ExternalInput")
    bv = nc.dram_tensor("bv", [CW, 1], f32, kind="ExternalInput")
    bp = nc.dram_tensor("bp", [1, C], f32, kind="ExternalInput")
    y = nc.dram_tensor("y", [SHARD, C], f32, kind="ExternalOutput")

    with tile.TileContext(nc) as tc:
        with (
            tc.tile_pool(name="const", bufs=1) as const,
            tc.tile_pool(name="dram", bufs=1, space="DRAM") as dram,
            tc.tile_pool(name="xtp", bufs=8) as xtp,
            tc.tile_pool(name="x8p", bufs=4) as x8p,
            tc.tile_pool(name="wqkv", bufs=1) as wqkvp,
            tc.tile_pool(name="qkv", bufs=1) as qkvp,
            tc.tile_pool(name="q8p", bufs=2) as q8p,
            tc.tile_pool(name="pt", bufs=4) as ptp,
            tc.tile_pool(name="otp", bufs=1) as otp,
            tc.tile_pool(name="sm", bufs=1) as smp,
            tc.tile_pool(name="proj", bufs=1) as projp,
            tc.tile_pool(name="ysb", bufs=1) as ysbp,
        ):
            # ---- collective warm-up (channel init overlaps compute) ----
            warm_i = dram.tile([8, 16], f32, name="warm_i")
            warm_o = dram.tile([8, 16], f32, name="warm_o")
            wtile = const.tile([8, 16], f32, name="wtile")
            nc.vector.memset(wtile[:], 0.0)
            nc.sync.dma_start(warm_i[:], wtile[:])
            nc.gpsimd.collective_compute(
                "AllToAll", mybir.AluOpType.bypass,
                replica_groups=[list(range(NCORES))],
                ins=[warm_i[:].opt()], outs=[warm_o[:].opt()],
            )

            # ---- weights + x, in first-use order ----
            wq_sb = wqkvp.tile([128, KT * CW], f8, name="wq_sb")
            nc.sync.dma_start(wq_sb[:], wq[:])
            x8_tiles = {}
            x8t = x8p.tile([128, KT * 512], f8, name="x8_0_0", tag="x8")
            nc.sync.dma_start(
                x8t[:].rearrange("p (a t) -> p a t", a=KT),
                x8[0].rearrange("p (a t) -> p a t", a=KT)[:, :, 0:512])
            x8_tiles[(0, 0)] = x8t
            wk_sb = wqkvp.tile([128, KT * CW], f8, name="wk_sb")
            wv_sb = wqkvp.tile([128, KT * CW], f16, name="wv_sb")
            nc.sync.dma_start(wk_sb[:], wk[:])
            nc.sync.dma_start(wv_sb[:], wv[:])

            bq_t = const.tile([CW, 1], f32, name="bq_t")
            bk_t = const.tile([CW, 1], f32, name="bk_t")
            nc.sync.dma_start(bq_t[:], bq[:])
            nc.sync.dma_start(bk_t[:], bk[:])
            bv_t = const.tile([CW, 1], f32, name="bv_t")
            nc.sync.dma_start(bv_t[:], bv[:])
            bp_row = const.tile([1, C], f32, name="bp_row")
            nc.sync.dma_start(bp_row[:], bp[:])

            for b in range(B):
                for j in range(4):
                    if (b, j) == (0, 0):
                        continue
                    x8t = x8p.tile([128, KT * 512], f8, name=f"x8_{b}_{j}",
                                   tag="x8")
                    nc.sync.dma_start(
                        x8t[:].rearrange("p (a t) -> p a t", a=KT),
                        x8[b].rearrange("p (a t) -> p a t", a=KT)[
                            :, :, 512 * j : 512 * (j + 1)])
                    x8_tiles[(b, j)] = x8t
            xt_tiles = {}
            for b in range(B):
                for j in range(4):
                    xt = xtp.tile([128, KT * 512], f16, name=f"xt{b}_{j}",
                                  tag="xt")
                    nc.sync.dma_start(
                        xt[:].rearrange("p (a t) -> p a t", a=KT),
                        xT[b].rearrange("p (a t) -> p a t", a=KT)[
                            :, :, 512 * j : 512 * (j + 1)])
                    xt_tiles[(b, j)] = xt
            # wp after x so its 2MB never delays the x stream
            wp_sb = projp.tile([128, KT * C], f16, name="wp_sb")
            nc.sync.dma_start(wp_sb[:], wp[:])

            # ---- constants ----
            trimask = const.tile([128, 128], f16, name="trimask")
            make_upper_triangular(nc, trimask[:], val=1.0, diag=True)
            ident = const.tile([128, 128], f16, name="ident")
            make_identity(nc, ident[:])
            bpb = const.tile([128, C], f32, name="bpb")
            nc.gpsimd.partition_broadcast(bpb[:], bp_row[:])

            # ---- psum pools ----
            qkv_psum = tc.tile_pool(name="psqk", bufs=2, space="PSUM")
            psqk = qkv_psum.__enter__()
            # PE warm-up: dummy matmuls on the (small, early) weight tile keep
            # the HAM activity monitor at full clock while x streams in.
            warm_ps = psqk.tile([128, 512], f32, name="warm_ps", tag="ps_qk")
            for _ in range(16):
                nc.tensor.matmul(
                    warm_ps[:], wq_sb[:, 0:CW], wq_sb[:, 0:512],
                    start=True, stop=True,
                )
            nc.vector.memset(warm_ps[:, 0:2], 0.0)
            attn_psum_s = tc.tile_pool(name="ps_s", bufs=2, space="PSUM")
            ps_s = attn_psum_s.__enter__()
            attn_psum_o = tc.tile_pool(name="ps_o", bufs=1, space="PSUM")
            ps_o = attn_psum_o.__enter__()

            qT8, kT8, v_sb, ot_sb, r_all = {}, {}, {}, {}, {}

            # a2a group g covers global rows [1024g, 1024(g+1)); core c owns
            # rows 1024g + 128c .. +128. Chunk (b, j) fills slots
            # 4*(j%2)+q of group 2b + j//2.
            a2a_in = [dram.tile([NCORES, 128, 128], f16, name=f"a2a_in{g}")
                      for g in range(4)]
            a2a_out = [dram.tile([NCORES, 128, 128], f16, name=f"a2a_out{g}")
                       for g in range(4)]

            def qkv_gen(b):
                """Generator emitting qkv(b) one small PE packet per yield,
                for interleaving into attention's ACT-bound bubbles."""
                qt_tmp = q8p.tile([128, T], f8, name=f"qtmp{b}", tag="qtmp",
                                  bufs=1)
                kt_tmp = q8p.tile([128, T], f8, name=f"ktmp{b}", tag="ktmp",
                                  bufs=1)
                # q,k: fp8 DoubleRow over 256-deep contraction pairs
                for dst, w_sb, bias in ((qt_tmp, wq_sb, bq_t),
                                        (kt_tmp, wk_sb, bk_t)):
                    for j in range(4):
                        ps = psqk.tile([128, 512], f32, name="ps_qk",
                                       tag="ps_qk")
                        x8t = x8_tiles[(b, j)]
                        for a2 in range(KT // 2):
                            nc.tensor.matmul(
                                ps[:],
                                w_sb[:, 2 * CW * a2 : 2 * CW * (a2 + 1)]
                                    .rearrange("p (two m) -> p two m", two=2),
                                x8t[:, 1024 * a2 : 1024 * (a2 + 1)]
                                    .rearrange("p (two t) -> p two t", two=2),
                                start=(a2 == 0), stop=(a2 == KT // 2 - 1),
                                perf_mode=mybir.MatmulPerfMode.DoubleRow,
                            )
                            yield
                        nc.vector.tensor_scalar_add(
                            dst[:, 512 * j : 512 * (j + 1)], ps[:], bias[:]
                        )
                        yield
                # repack into DoubleRow layout [32, 2T]: block i holds
                # head-dim rows [32i, 32i+32)
                for h in range(2):
                    q8 = q8p.tile([32, 2 * T], f8, name=f"q8_{b}{h}",
                                  tag=f"q8_{h}")
                    k8 = q8p.tile([32, 2 * T], f8, name=f"k8_{b}{h}",
                                  tag=f"k8_{h}")
                    for i in range(2):
                        s = slice(64 * h + 32 * i, 64 * h + 32 * (i + 1))
                        nc.sync.dma_start(q8[:, T * i : T * (i + 1)],
                                          qt_tmp[s, :])
                        nc.sync.dma_start(k8[:, T * i : T * (i + 1)],
                                          kt_tmp[s, :])
                    qT8[(b, h)], kT8[(b, h)] = q8, k8
                    yield
                # v: fp16
                vT_b = qkvp.tile([128, T], f16, name=f"vT{b}", tag="vT",
                                 bufs=1)
                for j in range(4):
                    ps = psqk.tile([128, 512], f32, name="ps_vT", tag="ps_qk")
                    for a in range(KT):
                        nc.tensor.matmul(
                            ps[:],
                            wv_sb[:, CW * a : CW * (a + 1)],
                            xt_tiles[(b, j)][:, 512 * a : 512 * (a + 1)],
                            start=(a == 0), stop=(a == KT - 1),
                        )
                        if a % 2 == 1:
                            yield
                    nc.vector.tensor_scalar_add(
                        vT_b[:, 512 * j : 512 * (j + 1)], ps[:], bv_t[:]
                    )
                    yield
                v_b = []
                for m in range(TT):
                    vt = qkvp.tile([128, 2 * (D + 1)], f16, name=f"v{b}_{m}")
                    tps = psqk.tile([128, 128], f16, name="ps_tr", tag="ps_qk")
                    nc.tensor.transpose(
                        tps[:], vT_b[:, 128 * m : 128 * (m + 1)], ident[:]
                    )
                    nc.vector.tensor_copy(
                        vt[:].rearrange("p (a m) -> p a m", a=2)[:, :, 0:D],
                        tps[:].rearrange("p (a m) -> p a m", a=2),
                    )
                    nc.vector.memset(vt[:, D : D + 1], 1.0)
                    nc.vector.memset(vt[:, 2 * D + 1 : 2 * D + 2], 1.0)
                    v_b.append(vt)
                    yield
                v_sb[b] = v_b
                ot = otp.tile([128, T], f16, name=f"ot{b}")
                ot_sb[b] = ot
                ra = smp.tile([1, 4096], f32, name=f"r_all{b}", tag="r_all",
                              bufs=1)
                r_all[b] = ra

            def proj_gen(g):
                """Generator emitting the group-g output projection one PE
                packet per yield."""
                ytg = projp.tile([128, C], f16, name=f"yt{g}", tag="ytg",
                                 bufs=2)
                nc.sync.dma_start(
                    ytg[:].rearrange("p (k t) -> p k t", k=KT),
                    a2a_out[g][:].rearrange("k p t -> p k t"),
                )
                yield
                ysb = ysbp.tile([128, C], f32, name="ysb", tag="ysb")
                for n in range(2):
                    ps = psqk.tile([128, 512], f32, name="ps_y", tag="ps_qk")
                    for k in range(KT):
                        nc.tensor.matmul(
                            ps[:],
                            ytg[:, 128 * k : 128 * (k + 1)],
                            wp_sb[:, C * k + 512 * n : C * k + 512 * (n + 1)],
                            start=(k == 0), stop=(k == KT - 1),
                        )
                        if k % 2 == 1:
                            yield
                    nc.vector.tensor_tensor(
                        ysb[:, 512 * n : 512 * (n + 1)],
                        ps[:],
                        bpb[:, 512 * n : 512 * (n + 1)],
                        op=mybir.AluOpType.add,
                    )
                    yield
                nc.sync.dma_start(y[128 * g : 128 * (g + 1), :], ysb[:])
                yield

            def chain(*gens):
                for g in gens:
                    yield from g

            def drain(filler):
                if filler is not None:
                    for _ in filler:
                        pass

            def emit_attn_chunk(b, j, filler=None, per_block=2):
                ot, ra = ot_sb[b], r_all[b]
                o_ps = [
                    ps_o.tile([65, 512], f32, name=f"o{h}", tag=f"o{h}")
                    for h in range(2)
                ]
                ilast = 4 * (j + 1) - 1
                for i in range(4 * (j + 1)):
                    off = max(0, 128 * i - 512 * j)
                    # one [128,1024] tile, head h in bank h
                    s_ps = ps_s.tile([128, 1024], f32, name="s_ps", tag="s")
                    pt = ptp.tile([128, 1024], f16, name="pt", tag="pt")
                    for h in range(2):
                        k8 = kT8[(b, h)].rearrange(
                            "p (two t) -> p two t", two=2)
                        q8 = qT8[(b, h)].rearrange(
                            "p (two t) -> p two t", two=2)
                        nc.tensor.matmul(
                            s_ps[:, 512 * h + off : 512 * (h + 1)],
                            k8[:, :, 128 * i : 128 * (i + 1)],
                            q8[:, :, 512 * j + off : 512 * (j + 1)],
                            start=True, stop=True,
                            perf_mode=mybir.MatmulPerfMode.DoubleRow,
                        )
                    # filler here: PE chews it while ScalarE runs the exp,
                    # so the PV below finds its pt ready
                    if filler is not None:
                        for _ in range(per_block):
                            if next(filler, _SENTINEL) is _SENTINEL:
                                break
                    nc.scalar.activation(
                        pt[:].rearrange("p (g w) -> p g w", g=2)[:, :, off:512],
                        s_ps[:].rearrange("p (g w) -> p g w", g=2)[:, :, off:512],
                        mybir.ActivationFunctionType.Exp,
                        scale=SCALE_S,
                    )
                    if 4 * j <= i:
                        for h in range(2):
                            nc.vector.tensor_tensor(
                                pt[:, 512 * h + off : 512 * h + off + 128],
                                pt[:, 512 * h + off : 512 * h + off + 128],
                                trimask[:],
                                op=mybir.AluOpType.mult,
                            )
                    for h in range(2):
                        nc.tensor.matmul(
                            o_ps[h][0:65, off:512],
                            v_sb[b][i][:, (D + 1) * h : (D + 1) * (h + 1)],
                            pt[:, 512 * h + off : 512 * (h + 1)],
                            start=(i == 0), stop=(i == ilast),
                        )
                # rowsums + undivided eviction for chunk j
                for h in range(2):
                    idx = 2 * j + h
                    nc.vector.tensor_copy(
                        ra[0:1, 512 * idx : 512 * (idx + 1)],
                        o_ps[h][64:65, :],
                    )
                    nc.vector.tensor_copy(
                        ot[64 * h : 64 * h + 64, 512 * j : 512 * (j + 1)],
                        o_ps[h][0:64, :],
                    )
                rs = ra[0:1, 1024 * j : 1024 * j + 1024]
                nc.vector.reciprocal_approx_fast(rs, rs)
                rb = smp.tile([128, 1024], f32, name="rb", tag="rb", bufs=2)
                nc.gpsimd.partition_broadcast(rb[:], rs)
                # rowsum multiply off DVE except on the final (tail) chunk
                meng = nc.vector if (b, j) == (1, 3) else nc.gpsimd
                for h in range(2):
                    sl = ot[64 * h : 64 * h + 64, 512 * j : 512 * (j + 1)]
                    meng.tensor_tensor(
                        sl, sl,
                        rb[64 * h : 64 * h + 64, 512 * h : 512 * (h + 1)],
                        op=mybir.AluOpType.mult,
                    )
                # stage into the a2a input for group 2b + j//2
                g = 2 * b + j // 2
                lo = 4 * (j % 2)
                nc.sync.dma_start(
                    a2a_in[g][lo : lo + 4].rearrange("q p t -> p q t"),
                    ot[:, 512 * j : 512 * (j + 1)].rearrange(
                        "p (q t) -> p q t", q=4),
                )

            def fire_group(g):
                nc.gpsimd.collective_compute(
                    "AllToAll", mybir.AluOpType.bypass,
                    replica_groups=[list(range(NCORES))],
                    ins=[a2a_in[g][:].opt()], outs=[a2a_out[g][:].opt()],
                )

            # qkv(0) runs straight (nothing to overlap with yet); qkv(1)
            # interleaves into attn(0)'s ACT-bound bubbles; the first three
            # output projections interleave into attn(1); proj(3) is the tail.
            drain(qkv_gen(0))
            f0 = qkv_gen(1)
            emit_attn_chunk(0, 0, f0, per_block=3)
            emit_attn_chunk(0, 1, f0, per_block=3)
            fire_group(0)
            emit_attn_chunk(0, 2, f0, per_block=3)
            emit_attn_chunk(0, 3, f0, per_block=3)
            fire_group(1)
            drain(f0)
            f1 = chain(proj_gen(0), proj_gen(1), proj_gen(2))
            emit_attn_chunk(1, 0, f1, per_block=1)
            emit_attn_chunk(1, 1, f1, per_block=1)
            fire_group(2)
            emit_attn_chunk(1, 2, f1, per_block=2)
            emit_attn_chunk(1, 3, f1, per_block=3)
            fire_group(3)
            drain(f1)
            drain(proj_gen(3))

            attn_psum_o.__exit__(None, None, None)
            attn_psum_s.__exit__(None, None, None)
            qkv_psum.__exit__(None, None, None)

    nc.compile()
    return nc


def _get_nc():
    if "nc" not in _CACHE:
        _CACHE["nc"] = _build_nc()
    return _CACHE["nc"]


def kernel(x, W_attn, b_attn, W_proj, b_proj, _trace=False):
    global LAST_EXEC_NS
    import ml_dtypes
    from concourse.bass_utils import run_bass_kernel_spmd

    f8np = ml_dtypes.float8_e4m3

    x = np.asarray(x, np.float32)
    W_attn = np.asarray(W_attn, np.float32)
    b_attn = np.asarray(b_attn, np.float32)
    W_proj = np.asarray(W_proj, np.float32)
    b_proj = np.asarray(b_proj, np.float32)

    def pmajor(w, dt):  # [C, M] -> [128, KT*M], k-tile a at cols [a*M:(a+1)*M]
        m = w.shape[1]
        return np.ascontiguousarray(
            w.reshape(KT, 128, m).transpose(1, 0, 2).reshape(128, KT * m)
        ).astype(dt)

    xT = np.transpose(x, (0, 2, 1))  # [B, C, T]
    xTp = np.ascontiguousarray(
        xT.reshape(B, KT, 128, T).transpose(0, 2, 1, 3).reshape(B, 128, KT * T)
    )
    xT16 = xTp.astype(np.float16)
    xT8 = xTp.astype(f8np)
    wp16 = pmajor(W_proj, np.float16)
    bp_h = np.ascontiguousarray(b_proj).reshape(1, C)

    in_maps = []
    for c in range(NCORES):
        s = slice(CW * c, CW * (c + 1))
        in_maps.append({
            "xT": xT16,
            "x8": xT8,
            "wq": pmajor(W_attn[:, s] * WSCALE, f8np),
            "wk": pmajor(W_attn[:, C:][:, s] * WSCALE, f8np),
            "wv": pmajor(W_attn[:, 2 * C:][:, s], np.float16),
            "wp": wp16,
            "bq": np.ascontiguousarray(b_attn[s] * WSCALE).reshape(CW, 1),
            "bk": np.ascontiguousarray(b_attn[C:][s] * WSCALE).reshape(CW, 1),
            "bv": np.ascontiguousarray(b_attn[2 * C:][s]).reshape(1, CW),
            "bp": bp_h,
        })

    nc = _get_nc()
    res = run_bass_kernel_spmd(nc, in_maps, list(range(NCORES)), trace=_trace)
    LAST_EXEC_NS = res.exec_time_ns

    out = np.empty((B, T, C), np.float32)
    for c in range(NCORES):
        for g in range(4):
            t0 = 1024 * (g % 2) + 128 * c
            out[g // 2, t0 : t0 + 128, :] = \
                res.results[c]["y"][128 * g : 128 * (g + 1)]
    return out


# revision 36
# speedup vs baseline: 1.0591x; 1.0591x over previous
"""Multi-head causal attention (B=2, T=2048, C=1024, H=16) on 8 trn2 NeuronCores.

Sharding: 2 heads per core (tensor-parallel over heads), both batch elements
on every core. Per core:
  1. qkv projection for its 2 heads. q^T,k^T run in fp8e4m3 DoubleRow mode
     (x and W_q/W_k supplied fp8 by the host, W pre-scaled by 32 to dodge
     fp8 denormals; the 1/1024 comes out in the exp scale). v runs in fp16.
     q^T,k^T are evicted to fp8 and repacked (SBUF->SBUF DMA) into DoubleRow
     layout [32, 2*T] so S = K^T Q also runs fp8 DoubleRow (2x PE rate).
  2. Flash-style causal attention in the S^T = k q^T layout: exp on ScalarE
     straight out of PSUM, row-sums folded into the P@V matmul via a ones
     column in V, reciprocal multiply on VectorE.
  3. The output exchange is split into 4 AllToAlls (one per 1024 global
     rows), each fired as soon as its two 512-query chunks finish; the
     per-group output projection is interleaved into the remaining
     attention compute. Core c owns rows 1024g+128c..+128 of group g.
Host side shards/transposes/casts inputs and reassembles the output.
"""

import sys

import numpy as np

if "/opt/trn_rl_repo" not in sys.path:
    sys.path.insert(0, "/opt/trn_rl_repo")

B, T, C, H, D = 2, 2048, 1024, 16, 64
NCORES = 8
HPC = H // NCORES          # heads per core = 2
CW = HPC * D               # per-core channel width = 128
KT = C // 128              # k tiles = 8
TT = T // 128              # t tiles = 16
SHARD = (B * T) // NCORES  # output rows per core = 512
SCALE = 1.0 / float(np.sqrt(C))

_CACHE = {}
LAST_EXEC_NS = None
_SENTINEL = object()


def _build_nc():
    import concourse.mybir as mybir
    import concourse.tile as tile
    from concourse import bacc
    from concourse.masks import make_identity, make_upper_triangular

    f32 = mybir.dt.float32
    f16 = mybir.dt.float16
    f8 = mybir.dt.float8e4

    nc = bacc.Bacc("TRN2", target_bir_lowering=False, debug=False,
                   num_devices=NCORES)

    xT = nc.dram_tensor("xT", [B, 128, KT * T], f16, kind="ExternalInput")
    wq = nc.dram_tensor("wq", [128, KT * CW], f16, kind="ExternalInput")
    wk = nc.dram_tensor("wk", [128, KT * CW], f16, kind="ExternalInput")
    wv = nc.dram_tensor("wv", [128, KT * CW], f16, kind="ExternalInput")
    wp = nc.dram_tensor("wp", [128, KT * C], f16, kind="ExternalInput")
    bq = nc.dram_tensor("bq", [CW, 1], f32, kind="ExternalInput")
    bk = nc.dram_tensor("bk", [CW, 1], f32, kind="ExternalInput")
    bv = nc.dram_tensor("bv", [CW, 1], f32, kind="ExternalInput")
    bp = nc.dram_tensor("bp", [1, C], f32, kind="ExternalInput")
    y = nc.dram_tensor("y", [SHARD, C], f32, kind="ExternalOutput")

    with tile.TileContext(nc) as tc:
        with (
            tc.tile_pool(name="const", bufs=1) as const,
            tc.tile_pool(name="dram", bufs=1, space="DRAM") as dram,
            tc.tile_pool(name="xtp", bufs=8) as xtp,
            tc.tile_pool(name="wqkv", bufs=1) as wqkvp,
            tc.tile_pool(name="qkv", bufs=1) as qkvp,
            tc.tile_pool(name="q8p", bufs=2) as q8p,
            tc.tile_pool(name="pt", bufs=4) as ptp,
            tc.tile_pool(name="otp", bufs=1) as otp,
            tc.tile_pool(name="sm", bufs=1) as smp,
            tc.tile_pool(name="proj", bufs=1) as projp,
            tc.tile_pool(name="ysb", bufs=2) as ysbp,
        ):
            # ---- collective warm-up (channel init overlaps compute) ----
            warm_i = dram.tile([8, 16], f32, name="warm_i")
            warm_o = dram.tile([8, 16], f32, name="warm_o")
            wtile = const.tile([8, 16], f32, name="wtile")
            nc.vector.memset(wtile[:], 0.0)
            nc.sync.dma_start(warm_i[:], wtile[:])
            nc.gpsimd.collective_compute(
                "AllToAll", mybir.AluOpType.bypass,
                replica_groups=[list(range(NCORES))],
                ins=[warm_i[:].opt()], outs=[warm_o[:].opt()],
            )

            # ---- weights + x, in first-use order ----
            wq_sb = wqkvp.tile([128, KT * CW], f16, name="wq_sb")
            nc.sync.dma_start(wq_sb[:], wq[:])
            wk_sb = wqkvp.tile([128, KT * CW], f16, name="wk_sb")
            wv_sb = wqkvp.tile([128, KT * CW], f16, name="wv_sb")
            nc.sync.dma_start(wk_sb[:], wk[:])
            nc.sync.dma_start(wv_sb[:], wv[:])

            bq_t = const.tile([CW, 1], f32, name="bq_t")
            bk_t = const.tile([CW, 1], f32, name="bk_t")
            nc.sync.dma_start(bq_t[:], bq[:])
            nc.sync.dma_start(bk_t[:], bk[:])
            bv_t = const.tile([CW, 1], f32, name="bv_t")
            nc.sync.dma_start(bv_t[:], bv[:])
            bp_row = const.tile([1, C], f32, name="bp_row")
            nc.sync.dma_start(bp_row[:], bp[:])

            xt_tiles = {}
            for b in range(B):
                for j in range(4):
                    xt = xtp.tile([128, KT * 512], f16, name=f"xt{b}_{j}",
                                  tag="xt")
                    nc.sync.dma_start(
                        xt[:].rearrange("p (a t) -> p a t", a=KT),
                        xT[b].rearrange("p (a t) -> p a t", a=KT)[
                            :, :, 512 * j : 512 * (j + 1)])
                    xt_tiles[(b, j)] = xt
            # wp after x so its 2MB never delays the x stream
            wp_sb = projp.tile([128, KT * C], f16, name="wp_sb")
            nc.sync.dma_start(wp_sb[:], wp[:])

            # ---- constants ----
            trimask = const.tile([128, 128], f16, name="trimask")
            make_upper_triangular(nc, trimask[:], val=1.0, diag=True)
            ident = const.tile([128, 128], f16, name="ident")
            make_identity(nc, ident[:])
            bpb = const.tile([128, C], f32, name="bpb")
            nc.gpsimd.partition_broadcast(bpb[:], bp_row[:])

            # ---- psum pools ----
            qkv_psum = tc.tile_pool(name="psqk", bufs=2, space="PSUM")
            psqk = qkv_psum.__enter__()
            # PE warm-up: dummy matmuls on the (small, early) weight tile keep
            # the HAM activity monitor at full clock while x streams in.
            warm_ps = psqk.tile([128, 512], f32, name="warm_ps", tag="ps_qk")
            for _ in range(16):
                nc.tensor.matmul(
                    warm_ps[:], wq_sb[:, 0:CW], wq_sb[:, 0:512],
                    start=True, stop=True,
                )
            nc.vector.memset(warm_ps[:, 0:2], 0.0)
            attn_psum_s = tc.tile_pool(name="ps_s", bufs=2, space="PSUM")
            ps_s = attn_psum_s.__enter__()
            attn_psum_o = tc.tile_pool(name="ps_o", bufs=1, space="PSUM")
            ps_o = attn_psum_o.__enter__()

            qT8, kT8, v_sb, ot_sb, r_all = {}, {}, {}, {}, {}

            # a2a group g covers global rows [1024g, 1024(g+1)); core c owns
            # rows 1024g + 128c .. +128. Chunk (b, j) fills slots
            # 4*(j%2)+q of group 2b + j//2.
            a2a_in = [dram.tile([NCORES, 128, 128], f16, name=f"a2a_in{g}")
                      for g in range(4)]
            a2a_out = [dram.tile([NCORES, 128, 128], f16, name=f"a2a_out{g}")
                       for g in range(4)]

            def qkv_gen(b):
                """Generator emitting qkv(b) one small PE packet per yield,
                for interleaving into attention's ACT-bound bubbles."""
                qt_tmp = q8p.tile([128, T], f8, name=f"qtmp{b}", tag="qtmp",
                                  bufs=1)
                kt_tmp = q8p.tile([128, T], f8, name=f"ktmp{b}", tag="ktmp",
                                  bufs=1)
                # q,k: fp16 matmuls, fp8 eviction
                for dst, w_sb, bias in ((qt_tmp, wq_sb, bq_t),
                                        (kt_tmp, wk_sb, bk_t)):
                    for j in range(4):
                        ps = psqk.tile([128, 512], f32, name="ps_qk",
                                       tag="ps_qk")
                        for a in range(KT):
                            nc.tensor.matmul(
                                ps[:],
                                w_sb[:, CW * a : CW * (a + 1)],
                                xt_tiles[(b, j)][:, 512 * a : 512 * (a + 1)],
                                start=(a == 0), stop=(a == KT - 1),
                            )
                            if a % 2 == 1:
                                yield
                        nc.vector.tensor_scalar_add(
                            dst[:, 512 * j : 512 * (j + 1)], ps[:], bias[:]
                        )
                        yield
                # repack into DoubleRow layout [32, 2T]: block i holds
                # head-dim rows [32i, 32i+32)
                for h in range(2):
                    q8 = q8p.tile([32, 2 * T], f8, name=f"q8_{b}{h}",
                                  tag=f"q8_{h}")
                    k8 = q8p.tile([32, 2 * T], f8, name=f"k8_{b}{h}",
                                  tag=f"k8_{h}")
                    for i in range(2):
                        s = slice(64 * h + 32 * i, 64 * h + 32 * (i + 1))
                        nc.sync.dma_start(q8[:, T * i : T * (i + 1)],
                                          qt_tmp[s, :])
                        nc.sync.dma_start(k8[:, T * i : T * (i + 1)],
                                          kt_tmp[s, :])
                    qT8[(b, h)], kT8[(b, h)] = q8, k8
                    yield
                # v: fp16
                vT_b = qkvp.tile([128, T], f16, name=f"vT{b}", tag="vT",
                                 bufs=1)
                for j in range(4):
                    ps = psqk.tile([128, 512], f32, name="ps_vT", tag="ps_qk")
                    for a in range(KT):
                        nc.tensor.matmul(
                            ps[:],
                            wv_sb[:, CW * a : CW * (a + 1)],
                            xt_tiles[(b, j)][:, 512 * a : 512 * (a + 1)],
                            start=(a == 0), stop=(a == KT - 1),
                        )
                        if a % 2 == 1:
                            yield
                    nc.vector.tensor_scalar_add(
                        vT_b[:, 512 * j : 512 * (j + 1)], ps[:], bv_t[:]
                    )
                    yield
                v_b = []
                for m in range(TT):
                    vt = qkvp.tile([128, 2 * (D + 1)], f16, name=f"v{b}_{m}")
                    tps = psqk.tile([128, 128], f16, name="ps_tr", tag="ps_qk")
                    nc.tensor.transpose(
                        tps[:], vT_b[:, 128 * m : 128 * (m + 1)], ident[:]
                    )
                    nc.vector.tensor_copy(
                        vt[:].rearrange("p (a m) -> p a m", a=2)[:, :, 0:D],
                        tps[:].rearrange("p (a m) -> p a m", a=2),
                    )
                    nc.vector.memset(vt[:, D : D + 1], 1.0)
                    nc.vector.memset(vt[:, 2 * D + 1 : 2 * D + 2], 1.0)
                    v_b.append(vt)
                    yield
                v_sb[b] = v_b
                ot = otp.tile([128, T], f16, name=f"ot{b}")
                ot_sb[b] = ot
                ra = smp.tile([1, 4096], f32, name=f"r_all{b}", tag="r_all",
                              bufs=1)
                r_all[b] = ra

            def proj_gen(g):
                """Generator emitting the group-g output projection one PE
                packet per yield."""
                ytg = projp.tile([128, C], f16, name=f"yt{g}", tag="ytg",
                                 bufs=2)
                nc.sync.dma_start(
                    ytg[:].rearrange("p (k t) -> p k t", k=KT),
                    a2a_out[g][:].rearrange("k p t -> p k t"),
                )
                yield
                ysb = ysbp.tile([128, C], f32, name="ysb", tag="ysb")
                for n in range(2):
                    ps = psqk.tile([128, 512], f32, name="ps_y", tag="ps_qk")
                    for k in range(KT):
                        nc.tensor.matmul(
                            ps[:],
                            ytg[:, 128 * k : 128 * (k + 1)],
                            wp_sb[:, C * k + 512 * n : C * k + 512 * (n + 1)],
                            start=(k == 0), stop=(k == KT - 1),
                        )
                        if k % 2 == 1:
                            yield
                    nc.vector.tensor_tensor(
                        ysb[:, 512 * n : 512 * (n + 1)],
                        ps[:],
                        bpb[:, 512 * n : 512 * (n + 1)],
                        op=mybir.AluOpType.add,
                    )
                    yield
                nc.sync.dma_start(y[128 * g : 128 * (g + 1), :], ysb[:])
                yield

            def chain(*gens):
                for g in gens:
                    yield from g

            def drain(filler):
                if filler is not None:
                    for _ in filler:
                        pass

            def emit_attn_chunk(b, j, filler=None, per_block=2):
                ot, ra = ot_sb[b], r_all[b]
                o_ps = [
                    ps_o.tile([65, 512], f32, name=f"o{h}", tag=f"o{h}")
                    for h in range(2)
                ]
                ilast = 4 * (j + 1) - 1
                for i in range(4 * (j + 1)):
                    off = max(0, 128 * i - 512 * j)
                    # one [128,1024] tile, head h in bank h
                    s_ps = ps_s.tile([128, 1024], f32, name="s_ps", tag="s")
                    pt = ptp.tile([128, 1024], f16, name="pt", tag="pt")
                    for h in range(2):
                        k8 = kT8[(b, h)].rearrange(
                            "p (two t) -> p two t", two=2)
                        q8 = qT8[(b, h)].rearrange(
                            "p (two t) -> p two t", two=2)
                        nc.tensor.matmul(
                            s_ps[:, 512 * h + off : 512 * (h + 1)],
                            k8[:, :, 128 * i : 128 * (i + 1)],
                            q8[:, :, 512 * j + off : 512 * (j + 1)],
                            start=True, stop=True,
                            perf_mode=mybir.MatmulPerfMode.DoubleRow,
                        )
                    # filler here: PE chews it while ScalarE runs the exp,
                    # so the PV below finds its pt ready
                    if filler is not None:
                        for _ in range(per_block):
                            if next(filler, _SENTINEL) is _SENTINEL:
                                break
                    nc.scalar.activation(
                        pt[:].rearrange("p (g w) -> p g w", g=2)[:, :, off:512],
                        s_ps[:].rearrange("p (g w) -> p g w", g=2)[:, :, off:512],
                        mybir.ActivationFunctionType.Exp,
                        scale=SCALE,
                    )
                    if 4 * j <= i:
                        for h in range(2):
                            nc.vector.tensor_tensor(
                                pt[:, 512 * h + off : 512 * h + off + 128],
                                pt[:, 512 * h + off : 512 * h + off + 128],
                                trimask[:],
                                op=mybir.AluOpType.mult,
                            )
                    for h in range(2):
                        nc.tensor.matmul(
                            o_ps[h][0:65, off:512],
                            v_sb[b][i][:, (D + 1) * h : (D + 1) * (h + 1)],
                            pt[:, 512 * h + off : 512 * (h + 1)],
                            start=(i == 0), stop=(i == ilast),
                        )
                # rowsums + undivided eviction for chunk j
                for h in range(2):
                    idx = 2 * j + h
                    nc.vector.tensor_copy(
                        ra[0:1, 512 * idx : 512 * (idx + 1)],
                        o_ps[h][64:65, :],
                    )
                    nc.vector.tensor_copy(
                        ot[64 * h : 64 * h + 64, 512 * j : 512 * (j + 1)],
                        o_ps[h][0:64, :],
                    )
                rs = ra[0:1, 1024 * j : 1024 * j + 1024]
                nc.vector.reciprocal_approx_fast(rs, rs)
                rb = smp.tile([128, 1024], f32, name="rb", tag="rb", bufs=2)
                nc.gpsimd.partition_broadcast(rb[:], rs)
                for h in range(2):
                    sl = ot[64 * h : 64 * h + 64, 512 * j : 512 * (j + 1)]
                    nc.vector.tensor_tensor(
                        sl, sl,
                        rb[64 * h : 64 * h + 64, 512 * h : 512 * (h + 1)],
                        op=mybir.AluOpType.mult,
                    )
                # stage into the a2a input for group 2b + j//2
                g = 2 * b + j // 2
                lo = 4 * (j % 2)
                nc.sync.dma_start(
                    a2a_in[g][lo : lo + 4].rearrange("q p t -> p q t"),
                    ot[:, 512 * j : 512 * (j + 1)].rearrange(
                        "p (q t) -> p q t", q=4),
                )

            def fire_group(g):
                nc.gpsimd.collective_compute(
                    "AllToAll", mybir.AluOpType.bypass,
                    replica_groups=[list(range(NCORES))],
                    ins=[a2a_in[g][:].opt()], outs=[a2a_out[g][:].opt()],
                )

            # qkv(0) runs straight (nothing to overlap with yet); qkv(1)
            # interleaves into attn(0)'s ACT-bound bubbles; the first three
            # output projections interleave into attn(1); proj(3) is the tail.
            drain(qkv_gen(0))
            f0 = qkv_gen(1)
            emit_attn_chunk(0, 0, f0, per_block=3)
            emit_attn_chunk(0, 1, f0, per_block=3)
            fire_group(0)
            emit_attn_chunk(0, 2, f0, per_block=2)
            emit_attn_chunk(0, 3, f0, per_block=2)
            fire_group(1)
            drain(f0)
            f1 = chain(proj_gen(0), proj_gen(1), proj_gen(2))
            emit_attn_chunk(1, 0, f1, per_block=1)
            emit_attn_chunk(1, 1, f1, per_block=1)
            fire_group(2)
            emit_attn_chunk(1, 2, f1, per_block=2)
            emit_attn_chunk(1, 3, f1, per_block=3)
            fire_group(3)
            drain(f1)
            drain(proj_gen(3))

            attn_psum_o.__exit__(None, None, None)
            attn_psum_s.__exit__(None, None, None)
            qkv_psum.__exit__(None, None, None)

    nc.compile()
    return nc


def _get_nc():
    if "nc" not in _CACHE:
        _CACHE["nc"] = _build_nc()
    return _CACHE["nc"]


def kernel(x, W_attn, b_attn, W_proj, b_proj, _trace=False):
    global LAST_EXEC_NS
    from concourse.bass_utils import run_bass_kernel_spmd

    x = np.asarray(x, np.float32)
    W_attn = np.asarray(W_attn, np.float32)
    b_attn = np.asarray(b_attn, np.float32)
    W_proj = np.asarray(W_proj, np.float32)
    b_proj = np.asarray(b_proj, np.float32)

    def pmajor(w, dt):  # [C, M] -> [128, KT*M], k-tile a at cols [a*M:(a+1)*M]
        m = w.shape[1]
        return np.ascontiguousarray(
            w.reshape(KT, 128, m).transpose(1, 0, 2).reshape(128, KT * m)
        ).astype(dt)

    xT = np.transpose(x, (0, 2, 1))  # [B, C, T]
    xT16 = np.ascontiguousarray(
        xT.reshape(B, KT, 128, T).transpose(0, 2, 1, 3).reshape(B, 128, KT * T)
    ).astype(np.float16)
    wp16 = pmajor(W_proj, np.float16)
    bp_h = np.ascontiguousarray(b_proj).reshape(1, C)

    in_maps = []
    for c in range(NCORES):
        s = slice(CW * c, CW * (c + 1))
        in_maps.append({
            "xT": xT16,
            "wq": pmajor(W_attn[:, s], np.float16),
            "wk": pmajor(W_attn[:, C:][:, s], np.float16),
            "wv": pmajor(W_attn[:, 2 * C:][:, s], np.float16),
            "wp": wp16,
            "bq": np.ascontiguousarray(b_attn[s]).reshape(CW, 1),
            "bk": np.ascontiguousarray(b_attn[C:][s]).reshape(CW, 1),
            "bv": np.ascontiguousarray(b_attn[2 * C:][s]).reshape(1, CW),
            "bp": bp_h,
        })

    nc = _get_nc()
    res = run_bass_kernel_spmd(nc, in_maps, list(range(NCORES)), trace=_trace)
    LAST_EXEC_NS = res.exec_time_ns

    out = np.empty((B, T, C), np.float32)
    for c in range(NCORES):
        for g in range(4):
            t0 = 1024 * (g % 2) + 128 * c
            out[g // 2, t0 : t0 + 128, :] = \
                res.results[c]["y"][128 * g : 128 * (g + 1)]
    return out
